# revision 29
# baseline (speedup 1.0000x reference)
"""Single-head attention (B=8, S=2048, D=1024, d_k=512), data-parallel over
batch across 8 NeuronCores, all-bf16 matmul pipeline.

Per-core dataflow (batch element b on core b), everything derived from x^T so
no on-chip transposes are needed anywhere:

  host:  xT = bf16(x[b].T)                            [1024, 2048]
  Q^T = Wq^T x + bq   -> [dk, S]  (k on partitions)   via matmul(lhsT=Wq, rhs=xT)
  K^T = Wk^T x + bk   -> [dk, S]
  V'  = x^T Wv + bv   -> [S, dk]  (s on partitions)   bv added on the DVE
                                                      eviction via a rank-1
                                                      ones (x) bv broadcast
  S^T[s,q] = (K^T)^T-free slices: matmul(lhsT=K^T tile, rhs=Q^T chunk)
  E^T = exp(S^T / sqrt(dk))       (no max subtraction; |scores| < ~4)
  Z[q] = ones^T @ (DVE running sum of E^T tiles)      [1, q]
  out[q,k] = sum_st matmul(lhsT=E^T tile, rhs=V' tile)  (q on partitions)
  out = bf16(pso * (1/Z))         one ACT op per q-block: per-partition
                                  scale AP; 1/Z via 4 PE transposes + DVE
                                  reciprocal on [128,4]. Since sum_s E/Z = 1
                                  the V'-folded bv comes out as out + bv.

Scheduling notes (measured on HW):
- each dma_start costs ~600ns serialized on its issuing sequencer, but one
  DMA is split across all 16 SDMA engines -> few, large DMAs on multiple
  queues (sync + scalar HWDGE rings, gpsimd SWDGE for the tiny biases).
- the first projection chunk runs at N=256 so the first matmul group only
  needs half of xt chunk 0 plus Wq m-blocks 0-1 (one DMA each).
- ~40 tiny warmup matmuls (gated only on a DVE memset) run during the
  initial DMA wait so the HAM clock gate is at 8/8 when the stream starts.
- bf16 streams at the same 1 column/cycle as fp32r but halves DMA bytes and
  LDWEIGHTS time (FWL works for 16-bit, not fp32).

Verified numerics vs fp32 reference: rel_err ~3.9e-3 (budget 2e-2).
"""

import numpy as np
import ml_dtypes

import concourse.bass as bass
import concourse.mybir as mybir
import concourse.tile as tile

B, S, D, DK = 8, 2048, 1024, 512
N_CORES = 8
P = 128
DT = D // P      # 8 d-tiles (contraction tiles for projections)
MT = DK // P     # 4 k-tiles
ST = S // P      # 16 s-tiles
NCH = S // 512   # 4 free-dim chunks of 512
SCALE = float(1.0 / np.sqrt(np.float32(DK)))
N_WARM = 112      # tiny PE warmup matmuls issued under the first DMA wait

F32 = mybir.dt.float32
F32R = mybir.dt.float32r
BF16 = mybir.dt.bfloat16
NPBF = ml_dtypes.bfloat16


def _round_f32r(a):
    """Round fp32 array to fp32r precision (rne at 11 mantissa bits), so the
    device can load it with a plain HWDGE DMA and bitcast to f32r."""
    u = np.ascontiguousarray(a, dtype=np.float32).view(np.uint32).astype(np.uint64)
    sh = np.uint64(12)
    half = np.uint64(1 << 11)
    lsb = (u >> sh) & np.uint64(1)
    r = ((u + half - np.uint64(1) + lsb) >> sh) << sh
    return r.astype(np.uint32).view(np.float32).reshape(a.shape)


def _split_excess_waits(nc, max_waits=1):
    """This walrus build accepts very few sync waits per instruction (and adds
    its own implicit queue waits to Drain). Move excess BIR waits onto
    dedicated NoOps inserted just before the over-subscribed instruction."""
    count = 0
    for f in nc.m.functions:
        for b in f.blocks:
            insts = list(b.instructions)
            out = []
            for ins in insts:
                si = getattr(ins, "sync_info", None)
                waits = list(si.on_wait) if si is not None else []
                cap = 0 if isinstance(ins, mybir.InstDrain) else max_waits
                if len(waits) > cap:
                    keep = waits[len(waits) - cap:] if cap else []
                    excess = waits[: len(waits) - cap]
                    for i in range(0, len(excess), max_waits):
                        chunk = excess[i : i + max_waits]
                        count += 1
                        nop = mybir.InstNoOp(
                            name=f"Wsplit-{count}", engine=ins.engine
                        )
                        nop.sync_info = mybir.SyncInfo(
                            on_wait=chunk, on_update=[]
                        )
                        out.append(nop)
                    ins.sync_info = mybir.SyncInfo(
                        on_wait=keep, on_update=list(si.on_update)
                    )
                out.append(ins)
            live = b.instructions
            live.clear()
            live.extend(out)
    return count


def build_nc(split_waits=True):
    nc = bass.Bass()
    xT = nc.dram_tensor("xT", [D, S], BF16, kind="ExternalInput")
    wq = nc.dram_tensor("wq", [D, DK], BF16, kind="ExternalInput")
    wk = nc.dram_tensor("wk", [D, DK], BF16, kind="ExternalInput")
    wv = nc.dram_tensor("wv", [D, DK], BF16, kind="ExternalInput")
    bq = nc.dram_tensor("bq", [P, MT], F32, kind="ExternalInput")
    bk = nc.dram_tensor("bk", [P, MT], F32, kind="ExternalInput")
    bvr = nc.dram_tensor("bvr", [1, DK], F32, kind="ExternalInput")
    out_d = nc.dram_tensor("out", [S, DK], BF16, kind="ExternalOutput")

    xT_r = xT.rearrange("(dt p) s -> p dt s", p=P)
    wq_r = wq.rearrange("(dt p) k -> p dt k", p=P)
    wk_r = wk.rearrange("(dt p) k -> p dt k", p=P)
    wv_r = wv.rearrange("(dt p) k -> p dt k", p=P)

    with tile.TileContext(nc) as tc:
        with tc.tile_pool(name="persist", bufs=1) as persist:
            qT = persist.tile([P, MT, S], BF16, tag="qT")
            kT = persist.tile([P, MT, S], BF16, tag="kT")
            v_sb = persist.tile([P, ST, DK], BF16, tag="v")
            bq_sb = persist.tile([P, MT], F32, tag="bq")
            bk_sb = persist.tile([P, MT], F32, tag="bk")
            bv_row = persist.tile([1, DK], F32R, tag="bv_row")
            bv_bc = persist.tile([P, DK], F32, tag="bv_bc")
            ones_col = persist.tile([P, 1], F32, tag="ones_col")
            ones_row = persist.tile([1, P], F32, tag="ones_row")
            ones_w = persist.tile([P, 64], F32, tag="ones_w")
            ones_id = persist.tile([1, 1], F32, tag="ones_id")

            nc.gpsimd.dma_start(out=bq_sb, in_=bq[:, :])
            nc.gpsimd.dma_start(out=bk_sb, in_=bk[:, :])
            nc.gpsimd.dma_start(out=bv_row, in_=bvr[:, :].bitcast(F32R))
            # ones via DVE memset only (no ACT dependency: ACT is busy with
            # its table load during the preamble); bitcast to f32r at use
            nc.vector.memset(ones_col, 1.0)
            nc.vector.memset(ones_row, 1.0)
            nc.vector.memset(ones_w, 1.0)
            nc.vector.memset(ones_id, 1.0)

            # ---------- Phase B: projections ----------
            # psC allocated first so it owns banks untouched by the phase-B
            # pools: phase C's first matmul then has no WAR dependency on
            # the last projection eviction
            psC_cm = tc.tile_pool(name="psC", bufs=3, space="PSUM")
            psC = psC_cm.__enter__()
            with tc.tile_pool(name="wpool", bufs=1) as wpool, \
                 tc.tile_pool(name="xpool", bufs=2) as xpool, \
                 tc.tile_pool(name="psW", bufs=1, space="PSUM") as psW:
                # PE warmup during the initial DMA wait: HAM reaches 8/8
                # before the real stream starts
                psw = psW.tile([1, 64], F32, tag="psw")
                for _ in range(N_WARM):
                    nc.tensor.matmul(
                        psw,
                        lhsT=ones_col[:, 0:1].bitcast(F32R),
                        rhs=ones_w.bitcast(F32R),
                        start=True,
                        stop=True,
                    )

                wq_sb = wpool.tile([P, DT, DK], BF16, tag="wq")
                wk_sb = wpool.tile([P, DT, DK], BF16, tag="wk")
                wv_sb = wpool.tile([P, DT, DK], BF16, tag="wv")
                xt0 = xpool.tile([P, DT, 512], BF16, tag="xt")
                # Large DMAs (1KB descriptors), arrival-ordered: scalar
                # ring carries wq, sync ring xt0 then wk/wv, so during the
                # head only (xt0, wq) compete for SDMA engines.
                nc.sync.dma_start(out=xt0[:, 0:4, :], in_=xT_r[:, 0:4, 0:512])
                nc.sync.dma_start(out=xt0[:, 4:8, :], in_=xT_r[:, 4:8, 0:512])
                nc.scalar.dma_start(out=wq_sb[:, 0:4, :], in_=wq_r[:, 0:4, :])
                nc.scalar.dma_start(out=wq_sb[:, 4:8, :], in_=wq_r[:, 4:8, :])
                for w_sb_, w_r_ in ((wk_sb, wk_r), (wv_sb, wv_r)):
                    nc.sync.dma_start(
                        out=w_sb_[:, 0:4, :], in_=w_r_[:, 0:4, :]
                    )
                    nc.sync.dma_start(
                        out=w_sb_[:, 4:8, :], in_=w_r_[:, 4:8, :]
                    )

                psB_cm = tc.tile_pool(name="psB", bufs=4, space="PSUM")
                psB = psB_cm.__enter__()

                def proj_group(wacc, m, xt, cols, out_sb, bias_sb, sc,
                               first=False):
                    ps = psB.tile([P, 512], F32, tag="psb")
                    ps_sl = ps[:, 0 : cols.stop - cols.start]
                    for d in range(DT):
                        if first and d == 4:
                            # second warmup burst inside the first group:
                            # covers the DMA wait for the second halves of
                            # xt0/wq so the HAM clock gate never sees idle
                            for _ in range(97):
                                nc.tensor.matmul(
                                    psw,
                                    lhsT=ones_col[:, 0:1].bitcast(F32R),
                                    rhs=ones_w.bitcast(F32R),
                                    start=True,
                                    stop=True,
                                )
                        nc.tensor.matmul(
                            ps_sl,
                            lhsT=wacc(m, d),
                            rhs=xt[:, d, cols],
                            start=(d == 0),
                            stop=(d == DT - 1),
                        )
                    nc.scalar.activation(
                        out=out_sb[
                            :, m, sc * 512 + cols.start : sc * 512 + cols.stop
                        ],
                        in_=ps_sl,
                        func=mybir.ActivationFunctionType.Identity,
                        bias=bias_sb[:, m : m + 1],
                    )

                def v_group(xt, i, sc):
                    st = sc * 4 + i
                    psv = psB.tile([P, 512], F32, tag="psb")
                    for d in range(DT):
                        nc.tensor.matmul(
                            psv,
                            lhsT=xt[:, d, i * P : (i + 1) * P],
                            rhs=wv_sb[:, d, :],
                            start=(d == 0),
                            stop=(d == DT - 1),
                        )
                    nc.vector.tensor_add(v_sb[:, st, :], psv, bv_bc)

                wq_a = lambda m, d: wq_sb[:, d, m * P : (m + 1) * P]
                wk_a = lambda m, d: wk_sb[:, d, m * P : (m + 1) * P]
                full = slice(0, 512)
                for sc in range(NCH):
                    if sc == 0:
                        xt = xt0
                    else:
                        xt = xpool.tile([P, DT, 512], BF16, tag="xt")
                        nc.sync.dma_start(
                            out=xt, in_=xT_r[:, :, sc * 512 : (sc + 1) * 512]
                        )
                    for m in range(MT):
                        proj_group(wq_a, m, xt, full, qT, bq_sb, sc,
                                   first=(sc == 0 and m == 0))
                        if sc == 0 and m == 0:
                            # bv broadcast [P, DK] via rank-1 ones (x)
                            # bv_row; placed after the first group so the
                            # PE never waits on the slow SWDGE bias DMA
                            psbv = psB.tile([P, 512], F32, tag="psb")
                            nc.tensor.matmul(
                                psbv,
                                lhsT=ones_row[0:1, :].bitcast(F32R),
                                rhs=bv_row[0:1, :],
                                start=True,
                                stop=True,
                            )
                            nc.scalar.copy(bv_bc, psbv)
                    for m in range(MT):
                        proj_group(wk_a, m, xt, full, kT, bk_sb, sc)
                    for i in range(4):
                        v_group(xt, i, sc)
                psB_cm.__exit__(None, None, None)

            # ---------- Phase C: attention ----------
            with tc.tile_pool(name="epool", bufs=2) as epool, \
                 tc.tile_pool(name="spool", bufs=2) as spool, \
                 tc.tile_pool(name="psO", bufs=2, space="PSUM") as psO, \
                 tc.tile_pool(name="psZ", bufs=1, space="PSUM") as psZ:
                for qc in range(NCH):
                    eT = epool.tile([P, ST, 512], BF16, tag="eT")
                    acc_z = spool.tile([P, 512], F32, tag="acc_z")
                    # S^T tiles: [s-part, 512 q], exp on eviction; running
                    # f32 sum of E^T tiles on DVE for the Z row
                    for st in range(ST):
                        pss = psC.tile([P, 512], F32, tag="pss")
                        for kt in range(MT):
                            nc.tensor.matmul(
                                pss,
                                lhsT=kT[:, kt, st * P : (st + 1) * P],
                                rhs=qT[:, kt, qc * 512 : (qc + 1) * 512],
                                start=(kt == 0),
                                stop=(kt == MT - 1),
                            )
                        nc.scalar.activation(
                            out=eT[:, st, :],
                            in_=pss,
                            func=mybir.ActivationFunctionType.Exp,
                            scale=SCALE,
                        )
                        if st == 0:
                            nc.vector.tensor_copy(acc_z, eT[:, 0, :])
                        else:
                            nc.vector.tensor_add(acc_z, acc_z, eT[:, st, :])
                    # Z chain on the side: reduce acc_z to [1,512], PE-
                    # transpose to q-partitions in 128-col strips, then one
                    # cheap full-width DVE reciprocal on [128,4]. The PE
                    # pieces are interleaved behind qsub 0's PV stream.
                    acc_zr = spool.tile([P, 512], F32R, tag="acc_zr")
                    nc.scalar.copy(acc_zr, acc_z)
                    psz = psZ.tile([1, 512], F32, tag="psz")
                    zrow = spool.tile([1, 512], F32, tag="zrow")
                    zcol_ps = psZ.tile([P, MT], F32, tag="zcol")
                    zrc = spool.tile([P, MT], F32, tag="zrc")

                    last = qc == NCH - 1
                    for j in range(MT):
                        nhalf = 2 if (last and j == MT - 1) else 1
                        for h in range(nhalf):
                            if nhalf == 1:
                                cols = slice(0, DK)
                            else:
                                cols = slice(0, 384) if h == 0 else slice(384, DK)
                            pso_full = psO.tile([P, 512], F32, tag="pso")
                            pso = pso_full[:, 0 : cols.stop - cols.start]
                            for st in range(ST):
                                nc.tensor.matmul(
                                    pso,
                                    lhsT=eT[:, st, j * P : (j + 1) * P],
                                    rhs=v_sb[:, st, cols],
                                    start=(st == 0),
                                    stop=(st == ST - 1),
                                )
                            if j == 0 and h == 0:
                                # Z reduce + transposes, scheduled behind
                                # qsub 0's PV group
                                nc.tensor.matmul(
                                    psz,
                                    lhsT=ones_col[:, 0:1].bitcast(F32R),
                                    rhs=acc_zr,
                                    start=True,
                                    stop=True,
                                )
                                nc.scalar.copy(zrow, psz[0:1, :])
                                for t in range(MT):
                                    nc.tensor.transpose(
                                        zcol_ps[:, t : t + 1],
                                        zrow[0:1, t * P : (t + 1) * P],
                                        ones_id[0:1, 0:1],
                                    )
                                nc.vector.reciprocal(zrc, zcol_ps)
                            ow = cols.stop - cols.start
                            oT = spool.tile([P, ow], BF16, tag=f"oT{ow}")
                            if nhalf == 2 and h == 1:
                                # tail piece: DVE is idle here and starts
                                # faster than ACT for the trailing chain
                                nc.vector.tensor_scalar_mul(
                                    oT, pso, zrc[:, j : j + 1]
                                )
                            else:
                                nc.scalar.mul(oT, pso, zrc[:, j : j + 1])
                            row0 = qc * 512 + j * P
                            if nhalf == 1:
                                nc.sync.dma_start(
                                    out=out_d[row0 : row0 + P, :], in_=oT
                                )
                            else:
                                # final store split across both HWDGE rings
                                # so only a short chain trails the last MM
                                nc.sync.dma_start(
                                    out=out_d[row0 : row0 + 64, cols],
                                    in_=oT[0:64, :],
                                )
                                nc.scalar.dma_start(
                                    out=out_d[row0 + 64 : row0 + P, cols],
                                    in_=oT[64:P, :],
                                )

            psC_cm.__exit__(None, None, None)

    if split_waits:
        _split_excess_waits(nc)
    return nc


_NC_CACHE = None


def _get_nc():
    global _NC_CACHE
    if _NC_CACHE is None:
        _NC_CACHE = build_nc()
    return _NC_CACHE


def _make_in_maps(x, Wq, bq, Wk, bk, Wv, bv):
    x = np.asarray(x, dtype=np.float32)
    Wq_b = np.ascontiguousarray(np.asarray(Wq, np.float32).astype(NPBF))
    Wk_b = np.ascontiguousarray(np.asarray(Wk, np.float32).astype(NPBF))
    Wv_b = np.ascontiguousarray(np.asarray(Wv, np.float32).astype(NPBF))
    bq_c = np.ascontiguousarray(np.asarray(bq, np.float32).reshape(MT, P).T)
    bk_c = np.ascontiguousarray(np.asarray(bk, np.float32).reshape(MT, P).T)
    bv_r = np.ascontiguousarray(
        _round_f32r(np.asarray(bv, np.float32)).reshape(1, DK)
    )
    in_maps = []
    for c in range(N_CORES):
        in_maps.append(
            {
                "xT": np.ascontiguousarray(x[c].T.astype(NPBF)),
                "wq": Wq_b,
                "wk": Wk_b,
                "wv": Wv_b,
                "bq": bq_c,
                "bk": bk_c,
                "bvr": bv_r,
            }
        )
    return in_maps


def run(x, Wq, bq, Wk, bk, Wv, bv, **run_kwargs):
    """Run on the 8 NeuronCores; returns (output, BassKernelResults)."""
    from concourse.bass_utils import run_bass_kernel_spmd

    nc = _get_nc()
    in_maps = _make_in_maps(x, Wq, bq, Wk, bk, Wv, bv)
    res = run_bass_kernel_spmd(
        nc, in_maps, core_ids=list(range(N_CORES)), **run_kwargs
    )
    out = np.stack(
        [np.asarray(r["out"]).astype(np.float32) for r in res.results],
        axis=0,
    )
    return out, res


def kernel(x, Wq, bq, Wk, bk, Wv, bv):
    out, _ = run(x, Wq, bq, Wk, bk, Wv, bv)
    return out


# revision 30
# speedup vs baseline: 1.0005x; 1.0005x over previous
"""Single-head attention (B=8, S=2048, D=1024, d_k=512), data-parallel over
batch across 8 NeuronCores, all-bf16 matmul pipeline.

Per-core dataflow (batch element b on core b), everything derived from x^T so
no on-chip transposes are needed anywhere:

  host:  xT = bf16(x[b].T)                            [1024, 2048]
  Q^T = Wq^T x + bq   -> [dk, S]  (k on partitions)   via matmul(lhsT=Wq, rhs=xT)
  K^T = Wk^T x + bk   -> [dk, S]
  V'  = x^T Wv + bv   -> [S, dk]  (s on partitions)   bv added on the DVE
                                                      eviction via a rank-1
                                                      ones (x) bv broadcast
  S^T[s,q] = (K^T)^T-free slices: matmul(lhsT=K^T tile, rhs=Q^T chunk)
  E^T = exp(S^T / sqrt(dk))       (no max subtraction; |scores| < ~4)
  Z[q] = ones^T @ (DVE running sum of E^T tiles)      [1, q]
  out[q,k] = sum_st matmul(lhsT=E^T tile, rhs=V' tile)  (q on partitions)
  out = bf16(pso * (1/Z))         one ACT op per q-block: per-partition
                                  scale AP; 1/Z via 4 PE transposes + DVE
                                  reciprocal on [128,4]. Since sum_s E/Z = 1
                                  the V'-folded bv comes out as out + bv.

Scheduling notes (measured on HW):
- each dma_start costs ~600ns serialized on its issuing sequencer, but one
  DMA is split across all 16 SDMA engines -> few, large DMAs on multiple
  queues (sync + scalar HWDGE rings, gpsimd SWDGE for the tiny biases).
- the first projection chunk runs at N=256 so the first matmul group only
  needs half of xt chunk 0 plus Wq m-blocks 0-1 (one DMA each).
- ~40 tiny warmup matmuls (gated only on a DVE memset) run during the
  initial DMA wait so the HAM clock gate is at 8/8 when the stream starts.
- bf16 streams at the same 1 column/cycle as fp32r but halves DMA bytes and
  LDWEIGHTS time (FWL works for 16-bit, not fp32).

Verified numerics vs fp32 reference: rel_err ~3.9e-3 (budget 2e-2).
"""

import numpy as np
import ml_dtypes

import concourse.bass as bass
import concourse.mybir as mybir
import concourse.tile as tile

B, S, D, DK = 8, 2048, 1024, 512
N_CORES = 8
P = 128
DT = D // P      # 8 d-tiles (contraction tiles for projections)
MT = DK // P     # 4 k-tiles
ST = S // P      # 16 s-tiles
NCH = S // 512   # 4 free-dim chunks of 512
SCALE = float(1.0 / np.sqrt(np.float32(DK)))
N_WARM = 112      # tiny PE warmup matmuls issued under the first DMA wait

F32 = mybir.dt.float32
F32R = mybir.dt.float32r
BF16 = mybir.dt.bfloat16
NPBF = ml_dtypes.bfloat16


def _round_f32r(a):
    """Round fp32 array to fp32r precision (rne at 11 mantissa bits), so the
    device can load it with a plain HWDGE DMA and bitcast to f32r."""
    u = np.ascontiguousarray(a, dtype=np.float32).view(np.uint32).astype(np.uint64)
    sh = np.uint64(12)
    half = np.uint64(1 << 11)
    lsb = (u >> sh) & np.uint64(1)
    r = ((u + half - np.uint64(1) + lsb) >> sh) << sh
    return r.astype(np.uint32).view(np.float32).reshape(a.shape)


def _split_excess_waits(nc, max_waits=1):
    """This walrus build accepts very few sync waits per instruction (and adds
    its own implicit queue waits to Drain). Move excess BIR waits onto
    dedicated NoOps inserted just before the over-subscribed instruction."""
    count = 0
    for f in nc.m.functions:
        for b in f.blocks:
            insts = list(b.instructions)
            out = []
            for ins in insts:
                si = getattr(ins, "sync_info", None)
                waits = list(si.on_wait) if si is not None else []
                cap = 0 if isinstance(ins, mybir.InstDrain) else max_waits
                if len(waits) > cap:
                    keep = waits[len(waits) - cap:] if cap else []
                    excess = waits[: len(waits) - cap]
                    for i in range(0, len(excess), max_waits):
                        chunk = excess[i : i + max_waits]
                        count += 1
                        nop = mybir.InstNoOp(
                            name=f"Wsplit-{count}", engine=ins.engine
                        )
                        nop.sync_info = mybir.SyncInfo(
                            on_wait=chunk, on_update=[]
                        )
                        out.append(nop)
                    ins.sync_info = mybir.SyncInfo(
                        on_wait=keep, on_update=list(si.on_update)
                    )
                out.append(ins)
            live = b.instructions
            live.clear()
            live.extend(out)
    return count


def build_nc(split_waits=True):
    nc = bass.Bass()
    xT = nc.dram_tensor("xT", [D, S], BF16, kind="ExternalInput")
    wq = nc.dram_tensor("wq", [D, DK], BF16, kind="ExternalInput")
    wk = nc.dram_tensor("wk", [D, DK], BF16, kind="ExternalInput")
    wv = nc.dram_tensor("wv", [D, DK], BF16, kind="ExternalInput")
    bq = nc.dram_tensor("bq", [P, MT], F32, kind="ExternalInput")
    bk = nc.dram_tensor("bk", [P, MT], F32, kind="ExternalInput")
    bvr = nc.dram_tensor("bvr", [1, DK], F32, kind="ExternalInput")
    out_d = nc.dram_tensor("out", [S, DK], BF16, kind="ExternalOutput")

    xT_r = xT.rearrange("(dt p) s -> p dt s", p=P)
    wq_r = wq.rearrange("(dt p) k -> p dt k", p=P)
    wk_r = wk.rearrange("(dt p) k -> p dt k", p=P)
    wv_r = wv.rearrange("(dt p) k -> p dt k", p=P)

    with tile.TileContext(nc) as tc:
        with tc.tile_pool(name="persist", bufs=1) as persist:
            qT = persist.tile([P, MT, S], BF16, tag="qT")
            kT = persist.tile([P, MT, S], BF16, tag="kT")
            v_sb = persist.tile([P, ST, DK], BF16, tag="v")
            bq_sb = persist.tile([P, MT], F32, tag="bq")
            bk_sb = persist.tile([P, MT], F32, tag="bk")
            bv_row = persist.tile([1, DK], F32R, tag="bv_row")
            bv_bc = persist.tile([P, DK], F32, tag="bv_bc")
            ones_col = persist.tile([P, 1], F32, tag="ones_col")
            ones_row = persist.tile([1, P], F32, tag="ones_row")
            ones_w = persist.tile([P, 64], F32, tag="ones_w")
            ones_id = persist.tile([1, 1], F32, tag="ones_id")

            nc.gpsimd.dma_start(out=bq_sb, in_=bq[:, :])
            nc.gpsimd.dma_start(out=bk_sb, in_=bk[:, :])
            nc.gpsimd.dma_start(out=bv_row, in_=bvr[:, :].bitcast(F32R))
            # ones via DVE memset only (no ACT dependency: ACT is busy with
            # its table load during the preamble); bitcast to f32r at use
            nc.vector.memset(ones_col, 1.0)
            nc.vector.memset(ones_row, 1.0)
            nc.vector.memset(ones_w, 1.0)
            nc.vector.memset(ones_id, 1.0)

            # ---------- Phase B: projections ----------
            # psC allocated first so it owns banks untouched by the phase-B
            # pools: phase C's first matmul then has no WAR dependency on
            # the last projection eviction
            psC_cm = tc.tile_pool(name="psC", bufs=3, space="PSUM")
            psC = psC_cm.__enter__()
            with tc.tile_pool(name="wpool", bufs=1) as wpool, \
                 tc.tile_pool(name="xpool", bufs=2) as xpool, \
                 tc.tile_pool(name="psW", bufs=1, space="PSUM") as psW:
                # PE warmup during the initial DMA wait: HAM reaches 8/8
                # before the real stream starts
                psw = psW.tile([1, 64], F32, tag="psw")
                for _ in range(N_WARM):
                    nc.tensor.matmul(
                        psw,
                        lhsT=ones_col[:, 0:1].bitcast(F32R),
                        rhs=ones_w.bitcast(F32R),
                        start=True,
                        stop=True,
                    )

                wq_sb = wpool.tile([P, DT, DK], BF16, tag="wq")
                wk_sb = wpool.tile([P, DT, DK], BF16, tag="wk")
                wv_sb = wpool.tile([P, DT, DK], BF16, tag="wv")
                xt0 = xpool.tile([P, DT, 512], BF16, tag="xt")
                # Large DMAs (1KB descriptors), arrival-ordered: scalar
                # ring carries wq, sync ring xt0 then wk/wv, so during the
                # head only (xt0, wq) compete for SDMA engines.
                nc.sync.dma_start(out=xt0[:, 0:4, :], in_=xT_r[:, 0:4, 0:512])
                nc.sync.dma_start(out=xt0[:, 4:8, :], in_=xT_r[:, 4:8, 0:512])
                nc.scalar.dma_start(out=wq_sb[:, 0:4, :], in_=wq_r[:, 0:4, :])
                nc.scalar.dma_start(out=wq_sb[:, 4:8, :], in_=wq_r[:, 4:8, :])
                for w_sb_, w_r_ in ((wk_sb, wk_r), (wv_sb, wv_r)):
                    nc.sync.dma_start(
                        out=w_sb_[:, 0:4, :], in_=w_r_[:, 0:4, :]
                    )
                    nc.sync.dma_start(
                        out=w_sb_[:, 4:8, :], in_=w_r_[:, 4:8, :]
                    )

                psB_cm = tc.tile_pool(name="psB", bufs=4, space="PSUM")
                psB = psB_cm.__enter__()

                def proj_group(wacc, m, xt, cols, out_sb, bias_sb, sc,
                               first=False):
                    ps = psB.tile([P, 512], F32, tag="psb")
                    ps_sl = ps[:, 0 : cols.stop - cols.start]
                    for d in range(DT):
                        if first and d == 4:
                            # second warmup burst inside the first group:
                            # covers the DMA wait for the second halves of
                            # xt0/wq so the HAM clock gate never sees idle
                            for _ in range(97):
                                nc.tensor.matmul(
                                    psw,
                                    lhsT=ones_col[:, 0:1].bitcast(F32R),
                                    rhs=ones_w.bitcast(F32R),
                                    start=True,
                                    stop=True,
                                )
                        nc.tensor.matmul(
                            ps_sl,
                            lhsT=wacc(m, d),
                            rhs=xt[:, d, cols],
                            start=(d == 0),
                            stop=(d == DT - 1),
                        )
                    nc.scalar.activation(
                        out=out_sb[
                            :, m, sc * 512 + cols.start : sc * 512 + cols.stop
                        ],
                        in_=ps_sl,
                        func=mybir.ActivationFunctionType.Identity,
                        bias=bias_sb[:, m : m + 1],
                    )

                def v_group(xt, i, sc):
                    st = sc * 4 + i
                    psv = psB.tile([P, 512], F32, tag="psb")
                    for d in range(DT):
                        nc.tensor.matmul(
                            psv,
                            lhsT=xt[:, d, i * P : (i + 1) * P],
                            rhs=wv_sb[:, d, :],
                            start=(d == 0),
                            stop=(d == DT - 1),
                        )
                    nc.vector.tensor_add(v_sb[:, st, :], psv, bv_bc)

                wq_a = lambda m, d: wq_sb[:, d, m * P : (m + 1) * P]
                wk_a = lambda m, d: wk_sb[:, d, m * P : (m + 1) * P]
                full = slice(0, 512)
                for sc in range(NCH):
                    if sc == 0:
                        xt = xt0
                    else:
                        xt = xpool.tile([P, DT, 512], BF16, tag="xt")
                        nc.sync.dma_start(
                            out=xt, in_=xT_r[:, :, sc * 512 : (sc + 1) * 512]
                        )
                    for m in range(MT):
                        proj_group(wq_a, m, xt, full, qT, bq_sb, sc,
                                   first=(sc == 0 and m == 0))
                        if sc == 0 and m == 0:
                            # bv broadcast [P, DK] via rank-1 ones (x)
                            # bv_row; placed after the first group so the
                            # PE never waits on the slow SWDGE bias DMA
                            psbv = psB.tile([P, 512], F32, tag="psb")
                            nc.tensor.matmul(
                                psbv,
                                lhsT=ones_row[0:1, :].bitcast(F32R),
                                rhs=bv_row[0:1, :],
                                start=True,
                                stop=True,
                            )
                            nc.scalar.copy(bv_bc, psbv)
                    for m in range(MT):
                        proj_group(wk_a, m, xt, full, kT, bk_sb, sc)
                    for i in range(4):
                        v_group(xt, i, sc)
                psB_cm.__exit__(None, None, None)

            # ---------- Phase C: attention ----------
            with tc.tile_pool(name="epool", bufs=2) as epool, \
                 tc.tile_pool(name="spool", bufs=2) as spool, \
                 tc.tile_pool(name="psO", bufs=2, space="PSUM") as psO, \
                 tc.tile_pool(name="psZ", bufs=1, space="PSUM") as psZ:
                for qc in range(NCH):
                    eT = epool.tile([P, ST, 512], BF16, tag="eT")
                    acc_z = spool.tile([P, 512], F32, tag="acc_z")
                    # S^T tiles: [s-part, 512 q], exp on eviction; running
                    # f32 sum of E^T tiles on DVE for the Z row
                    for st in range(ST):
                        pss = psC.tile([P, 512], F32, tag="pss")
                        for kt in range(MT):
                            nc.tensor.matmul(
                                pss,
                                lhsT=kT[:, kt, st * P : (st + 1) * P],
                                rhs=qT[:, kt, qc * 512 : (qc + 1) * 512],
                                start=(kt == 0),
                                stop=(kt == MT - 1),
                            )
                        nc.scalar.activation(
                            out=eT[:, st, :],
                            in_=pss,
                            func=mybir.ActivationFunctionType.Exp,
                            scale=SCALE,
                        )
                        if st == 0:
                            nc.vector.tensor_copy(acc_z, eT[:, 0, :])
                        else:
                            nc.vector.tensor_add(acc_z, acc_z, eT[:, st, :])
                    # Z chain on the side: reduce acc_z to [1,512], PE-
                    # transpose to q-partitions in 128-col strips, then one
                    # cheap full-width DVE reciprocal on [128,4]. The PE
                    # pieces are interleaved behind qsub 0's PV stream.
                    acc_zr = spool.tile([P, 512], F32R, tag="acc_zr")
                    nc.scalar.copy(acc_zr, acc_z)
                    psz = psZ.tile([1, 512], F32, tag="psz")
                    zrow = spool.tile([1, 512], F32, tag="zrow")
                    zcol_ps = psZ.tile([P, MT], F32, tag="zcol")
                    zrc = spool.tile([P, MT], F32, tag="zrc")

                    last = qc == NCH - 1
                    for j in range(MT):
                        nhalf = 2 if (last and j == MT - 1) else 1
                        for h in range(nhalf):
                            if nhalf == 1:
                                cols = slice(0, DK)
                            else:
                                cols = slice(0, 384) if h == 0 else slice(384, DK)
                            pso_full = psO.tile([P, 512], F32, tag="pso")
                            pso = pso_full[:, 0 : cols.stop - cols.start]
                            for st in range(ST):
                                nc.tensor.matmul(
                                    pso,
                                    lhsT=eT[:, st, j * P : (j + 1) * P],
                                    rhs=v_sb[:, st, cols],
                                    start=(st == 0),
                                    stop=(st == ST - 1),
                                )
                            if j == 0 and h == 0:
                                # Z reduce + transposes, scheduled behind
                                # qsub 0's PV group
                                nc.tensor.matmul(
                                    psz,
                                    lhsT=ones_col[:, 0:1].bitcast(F32R),
                                    rhs=acc_zr,
                                    start=True,
                                    stop=True,
                                )
                                nc.scalar.copy(zrow, psz[0:1, :])
                                for t in range(MT):
                                    nc.tensor.transpose(
                                        zcol_ps[:, t : t + 1],
                                        zrow[0:1, t * P : (t + 1) * P],
                                        ones_id[0:1, 0:1],
                                    )
                                nc.vector.reciprocal(zrc, zcol_ps)
                            ow = cols.stop - cols.start
                            oT = spool.tile([P, ow], BF16, tag=f"oT{ow}")
                            nc.scalar.mul(oT, pso, zrc[:, j : j + 1])
                            row0 = qc * 512 + j * P
                            if nhalf == 1:
                                nc.sync.dma_start(
                                    out=out_d[row0 : row0 + P, :], in_=oT
                                )
                            else:
                                # final store split across both HWDGE rings
                                # so only a short chain trails the last MM
                                nc.sync.dma_start(
                                    out=out_d[row0 : row0 + 64, cols],
                                    in_=oT[0:64, :],
                                )
                                nc.scalar.dma_start(
                                    out=out_d[row0 + 64 : row0 + P, cols],
                                    in_=oT[64:P, :],
                                )

            psC_cm.__exit__(None, None, None)

    if split_waits:
        _split_excess_waits(nc)
    return nc


_NC_CACHE = None


def _get_nc():
    global _NC_CACHE
    if _NC_CACHE is None:
        _NC_CACHE = build_nc()
    return _NC_CACHE


def _make_in_maps(x, Wq, bq, Wk, bk, Wv, bv):
    x = np.asarray(x, dtype=np.float32)
    Wq_b = np.ascontiguousarray(np.asarray(Wq, np.float32).astype(NPBF))
    Wk_b = np.ascontiguousarray(np.asarray(Wk, np.float32).astype(NPBF))
    Wv_b = np.ascontiguousarray(np.asarray(Wv, np.float32).astype(NPBF))
    bq_c = np.ascontiguousarray(np.asarray(bq, np.float32).reshape(MT, P).T)
    bk_c = np.ascontiguousarray(np.asarray(bk, np.float32).reshape(MT, P).T)
    bv_r = np.ascontiguousarray(
        _round_f32r(np.asarray(bv, np.float32)).reshape(1, DK)
    )
    in_maps = []
    for c in range(N_CORES):
        in_maps.append(
            {
                "xT": np.ascontiguousarray(x[c].T.astype(NPBF)),
                "wq": Wq_b,
                "wk": Wk_b,
                "wv": Wv_b,
                "bq": bq_c,
                "bk": bk_c,
                "bvr": bv_r,
            }
        )
    return in_maps


def run(x, Wq, bq, Wk, bk, Wv, bv, **run_kwargs):
    """Run on the 8 NeuronCores; returns (output, BassKernelResults)."""
    from concourse.bass_utils import run_bass_kernel_spmd

    nc = _get_nc()
    in_maps = _make_in_maps(x, Wq, bq, Wk, bk, Wv, bv)
    res = run_bass_kernel_spmd(
        nc, in_maps, core_ids=list(range(N_CORES)), **run_kwargs
    )
    out = np.stack(
        [np.asarray(r["out"]).astype(np.float32) for r in res.results],
        axis=0,
    )
    return out, res


def kernel(x, Wq, bq, Wk, bk, Wv, bv):
    out, _ = run(x, Wq, bq, Wk, bk, Wv, bv)
    return out


# revision 31
# speedup vs baseline: 1.0011x; 1.0006x over previous
"""Single-head attention (B=8, S=2048, D=1024, d_k=512), data-parallel over
batch across 8 NeuronCores, all-bf16 matmul pipeline.

Per-core dataflow (batch element b on core b), everything derived from x^T so
no on-chip transposes are needed anywhere:

  host:  xT = bf16(x[b].T)                            [1024, 2048]
  Q^T = Wq^T x + bq   -> [dk, S]  (k on partitions)   via matmul(lhsT=Wq, rhs=xT)
  K^T = Wk^T x + bk   -> [dk, S]
  V'  = x^T Wv + bv   -> [S, dk]  (s on partitions)   bv added on the DVE
                                                      eviction via a rank-1
                                                      ones (x) bv broadcast
  S^T[s,q] = (K^T)^T-free slices: matmul(lhsT=K^T tile, rhs=Q^T chunk)
  E^T = exp(S^T / sqrt(dk))       (no max subtraction; |scores| < ~4)
  Z[q] = ones^T @ (DVE running sum of E^T tiles)      [1, q]
  out[q,k] = sum_st matmul(lhsT=E^T tile, rhs=V' tile)  (q on partitions)
  out = bf16(pso * (1/Z))         one ACT op per q-block: per-partition
                                  scale AP; 1/Z via 4 PE transposes + DVE
                                  reciprocal on [128,4]. Since sum_s E/Z = 1
                                  the V'-folded bv comes out as out + bv.

Scheduling notes (measured on HW):
- each dma_start costs ~600ns serialized on its issuing sequencer, but one
  DMA is split across all 16 SDMA engines -> few, large DMAs on multiple
  queues (sync + scalar HWDGE rings, gpsimd SWDGE for the tiny biases).
- the first projection chunk runs at N=256 so the first matmul group only
  needs half of xt chunk 0 plus Wq m-blocks 0-1 (one DMA each).
- ~40 tiny warmup matmuls (gated only on a DVE memset) run during the
  initial DMA wait so the HAM clock gate is at 8/8 when the stream starts.
- bf16 streams at the same 1 column/cycle as fp32r but halves DMA bytes and
  LDWEIGHTS time (FWL works for 16-bit, not fp32).

Verified numerics vs fp32 reference: rel_err ~3.9e-3 (budget 2e-2).
"""

import numpy as np
import ml_dtypes

import concourse.bass as bass
import concourse.mybir as mybir
import concourse.tile as tile

B, S, D, DK = 8, 2048, 1024, 512
N_CORES = 8
P = 128
DT = D // P      # 8 d-tiles (contraction tiles for projections)
MT = DK // P     # 4 k-tiles
ST = S // P      # 16 s-tiles
NCH = S // 512   # 4 free-dim chunks of 512
SCALE = float(1.0 / np.sqrt(np.float32(DK)))
N_WARM = 112      # tiny PE warmup matmuls issued under the first DMA wait

F32 = mybir.dt.float32
F32R = mybir.dt.float32r
BF16 = mybir.dt.bfloat16
NPBF = ml_dtypes.bfloat16


def _round_f32r(a):
    """Round fp32 array to fp32r precision (rne at 11 mantissa bits), so the
    device can load it with a plain HWDGE DMA and bitcast to f32r."""
    u = np.ascontiguousarray(a, dtype=np.float32).view(np.uint32).astype(np.uint64)
    sh = np.uint64(12)
    half = np.uint64(1 << 11)
    lsb = (u >> sh) & np.uint64(1)
    r = ((u + half - np.uint64(1) + lsb) >> sh) << sh
    return r.astype(np.uint32).view(np.float32).reshape(a.shape)


def _split_excess_waits(nc, max_waits=1):
    """This walrus build accepts very few sync waits per instruction (and adds
    its own implicit queue waits to Drain). Move excess BIR waits onto
    dedicated NoOps inserted just before the over-subscribed instruction."""
    count = 0
    for f in nc.m.functions:
        for b in f.blocks:
            insts = list(b.instructions)
            out = []
            for ins in insts:
                si = getattr(ins, "sync_info", None)
                waits = list(si.on_wait) if si is not None else []
                cap = 0 if isinstance(ins, mybir.InstDrain) else max_waits
                if len(waits) > cap:
                    keep = waits[len(waits) - cap:] if cap else []
                    excess = waits[: len(waits) - cap]
                    for i in range(0, len(excess), max_waits):
                        chunk = excess[i : i + max_waits]
                        count += 1
                        nop = mybir.InstNoOp(
                            name=f"Wsplit-{count}", engine=ins.engine
                        )
                        nop.sync_info = mybir.SyncInfo(
                            on_wait=chunk, on_update=[]
                        )
                        out.append(nop)
                    ins.sync_info = mybir.SyncInfo(
                        on_wait=keep, on_update=list(si.on_update)
                    )
                out.append(ins)
            live = b.instructions
            live.clear()
            live.extend(out)
    return count


def build_nc(split_waits=True):
    nc = bass.Bass()
    xT = nc.dram_tensor("xT", [D, S], BF16, kind="ExternalInput")
    wq = nc.dram_tensor("wq", [D, DK], BF16, kind="ExternalInput")
    wk = nc.dram_tensor("wk", [D, DK], BF16, kind="ExternalInput")
    wv = nc.dram_tensor("wv", [D, DK], BF16, kind="ExternalInput")
    bq = nc.dram_tensor("bq", [P, MT], F32, kind="ExternalInput")
    bk = nc.dram_tensor("bk", [P, MT], F32, kind="ExternalInput")
    bvr = nc.dram_tensor("bvr", [1, DK], F32, kind="ExternalInput")
    out_d = nc.dram_tensor("out", [S, DK], BF16, kind="ExternalOutput")

    xT_r = xT.rearrange("(dt p) s -> p dt s", p=P)
    wq_r = wq.rearrange("(dt p) k -> p dt k", p=P)
    wk_r = wk.rearrange("(dt p) k -> p dt k", p=P)
    wv_r = wv.rearrange("(dt p) k -> p dt k", p=P)

    with tile.TileContext(nc) as tc:
        with tc.tile_pool(name="persist", bufs=1) as persist:
            qT = persist.tile([P, MT, S], BF16, tag="qT")
            kT = persist.tile([P, MT, S], BF16, tag="kT")
            v_sb = persist.tile([P, ST, DK], BF16, tag="v")
            bq_sb = persist.tile([P, MT], F32, tag="bq")
            bk_sb = persist.tile([P, MT], F32, tag="bk")
            bv_row = persist.tile([1, DK], F32R, tag="bv_row")
            bv_bc = persist.tile([P, DK], F32, tag="bv_bc")
            ones_col = persist.tile([P, 1], F32, tag="ones_col")
            ones_row = persist.tile([1, P], F32, tag="ones_row")
            ones_w = persist.tile([P, 64], F32, tag="ones_w")
            ones_id = persist.tile([1, 1], F32, tag="ones_id")

            nc.gpsimd.dma_start(out=bq_sb, in_=bq[:, :])
            nc.gpsimd.dma_start(out=bk_sb, in_=bk[:, :])
            nc.gpsimd.dma_start(out=bv_row, in_=bvr[:, :].bitcast(F32R))
            # ones via DVE memset only (no ACT dependency: ACT is busy with
            # its table load during the preamble); bitcast to f32r at use
            nc.vector.memset(ones_col, 1.0)
            nc.vector.memset(ones_row, 1.0)
            nc.vector.memset(ones_w, 1.0)
            nc.vector.memset(ones_id, 1.0)

            # ---------- Phase B: projections ----------
            # psC allocated first so it owns banks untouched by the phase-B
            # pools: phase C's first matmul then has no WAR dependency on
            # the last projection eviction
            psC_cm = tc.tile_pool(name="psC", bufs=3, space="PSUM")
            psC = psC_cm.__enter__()
            with tc.tile_pool(name="wpool", bufs=1) as wpool, \
                 tc.tile_pool(name="xpool", bufs=2) as xpool, \
                 tc.tile_pool(name="psW", bufs=1, space="PSUM") as psW:
                # PE warmup during the initial DMA wait: HAM reaches 8/8
                # before the real stream starts
                psw = psW.tile([1, 64], F32, tag="psw")
                for _ in range(N_WARM):
                    nc.tensor.matmul(
                        psw,
                        lhsT=ones_col[:, 0:1].bitcast(F32R),
                        rhs=ones_w.bitcast(F32R),
                        start=True,
                        stop=True,
                    )

                wq_sb = wpool.tile([P, DT, DK], BF16, tag="wq")
                wk_sb = wpool.tile([P, DT, DK], BF16, tag="wk")
                wv_sb = wpool.tile([P, DT, DK], BF16, tag="wv")
                xt0 = xpool.tile([P, DT, 512], BF16, tag="xt")
                # Large DMAs (1KB descriptors), arrival-ordered: scalar
                # ring carries wq, sync ring xt0 then wk/wv, so during the
                # head only (xt0, wq) compete for SDMA engines.
                nc.sync.dma_start(out=xt0[:, 0:4, :], in_=xT_r[:, 0:4, 0:512])
                nc.sync.dma_start(out=xt0[:, 4:8, :], in_=xT_r[:, 4:8, 0:512])
                nc.scalar.dma_start(out=wq_sb[:, 0:4, :], in_=wq_r[:, 0:4, :])
                nc.scalar.dma_start(out=wq_sb[:, 4:8, :], in_=wq_r[:, 4:8, :])
                for w_sb_, w_r_ in ((wk_sb, wk_r), (wv_sb, wv_r)):
                    nc.sync.dma_start(
                        out=w_sb_[:, 0:4, :], in_=w_r_[:, 0:4, :]
                    )
                    nc.sync.dma_start(
                        out=w_sb_[:, 4:8, :], in_=w_r_[:, 4:8, :]
                    )

                psB_cm = tc.tile_pool(name="psB", bufs=4, space="PSUM")
                psB = psB_cm.__enter__()

                def proj_group(wacc, m, xt, cols, out_sb, bias_sb, sc,
                               first=False):
                    ps = psB.tile([P, 512], F32, tag="psb")
                    ps_sl = ps[:, 0 : cols.stop - cols.start]
                    for d in range(DT):
                        if first and d == 4:
                            # second warmup burst inside the first group:
                            # covers the DMA wait for the second halves of
                            # xt0/wq so the HAM clock gate never sees idle
                            for _ in range(97):
                                nc.tensor.matmul(
                                    psw,
                                    lhsT=ones_col[:, 0:1].bitcast(F32R),
                                    rhs=ones_w.bitcast(F32R),
                                    start=True,
                                    stop=True,
                                )
                        nc.tensor.matmul(
                            ps_sl,
                            lhsT=wacc(m, d),
                            rhs=xt[:, d, cols],
                            start=(d == 0),
                            stop=(d == DT - 1),
                        )
                    nc.scalar.activation(
                        out=out_sb[
                            :, m, sc * 512 + cols.start : sc * 512 + cols.stop
                        ],
                        in_=ps_sl,
                        func=mybir.ActivationFunctionType.Identity,
                        bias=bias_sb[:, m : m + 1],
                    )

                def v_group(xt, i, sc):
                    st = sc * 4 + i
                    psv = psB.tile([P, 512], F32, tag="psb")
                    for d in range(DT):
                        nc.tensor.matmul(
                            psv,
                            lhsT=xt[:, d, i * P : (i + 1) * P],
                            rhs=wv_sb[:, d, :],
                            start=(d == 0),
                            stop=(d == DT - 1),
                        )
                    nc.vector.tensor_add(v_sb[:, st, :], psv, bv_bc)

                wq_a = lambda m, d: wq_sb[:, d, m * P : (m + 1) * P]
                wk_a = lambda m, d: wk_sb[:, d, m * P : (m + 1) * P]
                full = slice(0, 512)
                for sc in range(NCH):
                    if sc == 0:
                        xt = xt0
                    else:
                        xt = xpool.tile([P, DT, 512], BF16, tag="xt")
                        nc.sync.dma_start(
                            out=xt, in_=xT_r[:, :, sc * 512 : (sc + 1) * 512]
                        )
                    for m in range(MT):
                        proj_group(wq_a, m, xt, full, qT, bq_sb, sc,
                                   first=(sc == 0 and m == 0))
                        if sc == 0 and m == 0:
                            # bv broadcast [P, DK] via rank-1 ones (x)
                            # bv_row; placed after the first group so the
                            # PE never waits on the slow SWDGE bias DMA
                            psbv = psB.tile([P, 512], F32, tag="psb")
                            nc.tensor.matmul(
                                psbv,
                                lhsT=ones_row[0:1, :].bitcast(F32R),
                                rhs=bv_row[0:1, :],
                                start=True,
                                stop=True,
                            )
                            nc.scalar.copy(bv_bc, psbv)
                    for m in range(MT):
                        proj_group(wk_a, m, xt, full, kT, bk_sb, sc)
                    for i in range(4):
                        v_group(xt, i, sc)
                psB_cm.__exit__(None, None, None)

            # ---------- Phase C: attention ----------
            with tc.tile_pool(name="epool", bufs=2) as epool, \
                 tc.tile_pool(name="spool", bufs=2) as spool, \
                 tc.tile_pool(name="psO", bufs=2, space="PSUM") as psO, \
                 tc.tile_pool(name="psZ", bufs=1, space="PSUM") as psZ:
                for qc in range(NCH):
                    eT = epool.tile([P, ST, 512], BF16, tag="eT")
                    acc_z = spool.tile([P, 512], F32R, tag="acc_z")
                    # S^T tiles: [s-part, 512 q], exp on eviction; running
                    # f32 sum of E^T tiles on DVE for the Z row
                    for st in range(ST):
                        pss = psC.tile([P, 512], F32, tag="pss")
                        for kt in range(MT):
                            nc.tensor.matmul(
                                pss,
                                lhsT=kT[:, kt, st * P : (st + 1) * P],
                                rhs=qT[:, kt, qc * 512 : (qc + 1) * 512],
                                start=(kt == 0),
                                stop=(kt == MT - 1),
                            )
                        nc.scalar.activation(
                            out=eT[:, st, :],
                            in_=pss,
                            func=mybir.ActivationFunctionType.Exp,
                            scale=SCALE,
                        )
                        if st == 0:
                            nc.vector.tensor_copy(acc_z, eT[:, 0, :])
                        else:
                            nc.vector.tensor_add(acc_z, acc_z, eT[:, st, :])
                    # Z chain on the side: reduce acc_z to [1,512], PE-
                    # transpose to q-partitions in 128-col strips, then one
                    # cheap full-width DVE reciprocal on [128,4]. The PE
                    # pieces are interleaved behind qsub 0's PV stream.
                    psz = psZ.tile([1, 512], F32, tag="psz")
                    zrow = spool.tile([1, 512], F32, tag="zrow")
                    zcol_ps = psZ.tile([P, MT], F32, tag="zcol")
                    zrc = spool.tile([P, MT], F32, tag="zrc")

                    last = qc == NCH - 1
                    for j in range(MT):
                        nhalf = 2 if (last and j == MT - 1) else 1
                        for h in range(nhalf):
                            if nhalf == 1:
                                cols = slice(0, DK)
                            else:
                                cols = slice(0, 384) if h == 0 else slice(384, DK)
                            pso_full = psO.tile([P, 512], F32, tag="pso")
                            pso = pso_full[:, 0 : cols.stop - cols.start]
                            for st in range(ST):
                                nc.tensor.matmul(
                                    pso,
                                    lhsT=eT[:, st, j * P : (j + 1) * P],
                                    rhs=v_sb[:, st, cols],
                                    start=(st == 0),
                                    stop=(st == ST - 1),
                                )
                            if j == 0 and h == 0:
                                # Z reduce + transposes, scheduled behind
                                # qsub 0's PV group
                                nc.tensor.matmul(
                                    psz,
                                    lhsT=ones_col[:, 0:1].bitcast(F32R),
                                    rhs=acc_z,
                                    start=True,
                                    stop=True,
                                )
                                nc.scalar.copy(zrow, psz[0:1, :])
                                for t in range(MT):
                                    nc.tensor.transpose(
                                        zcol_ps[:, t : t + 1],
                                        zrow[0:1, t * P : (t + 1) * P],
                                        ones_id[0:1, 0:1],
                                    )
                                nc.vector.reciprocal(zrc, zcol_ps)
                            ow = cols.stop - cols.start
                            oT = spool.tile([P, ow], BF16, tag=f"oT{ow}")
                            nc.scalar.mul(oT, pso, zrc[:, j : j + 1])
                            row0 = qc * 512 + j * P
                            if nhalf == 1:
                                nc.sync.dma_start(
                                    out=out_d[row0 : row0 + P, :], in_=oT
                                )
                            else:
                                # final store split across both HWDGE rings
                                # so only a short chain trails the last MM
                                nc.sync.dma_start(
                                    out=out_d[row0 : row0 + 64, cols],
                                    in_=oT[0:64, :],
                                )
                                nc.scalar.dma_start(
                                    out=out_d[row0 + 64 : row0 + P, cols],
                                    in_=oT[64:P, :],
                                )

            psC_cm.__exit__(None, None, None)

    if split_waits:
        _split_excess_waits(nc)
    return nc


_NC_CACHE = None


def _get_nc():
    global _NC_CACHE
    if _NC_CACHE is None:
        _NC_CACHE = build_nc()
    return _NC_CACHE


def _make_in_maps(x, Wq, bq, Wk, bk, Wv, bv):
    x = np.asarray(x, dtype=np.float32)
    Wq_b = np.ascontiguousarray(np.asarray(Wq, np.float32).astype(NPBF))
    Wk_b = np.ascontiguousarray(np.asarray(Wk, np.float32).astype(NPBF))
    Wv_b = np.ascontiguousarray(np.asarray(Wv, np.float32).astype(NPBF))
    bq_c = np.ascontiguousarray(np.asarray(bq, np.float32).reshape(MT, P).T)
    bk_c = np.ascontiguousarray(np.asarray(bk, np.float32).reshape(MT, P).T)
    bv_r = np.ascontiguousarray(
        _round_f32r(np.asarray(bv, np.float32)).reshape(1, DK)
    )
    in_maps = []
    for c in range(N_CORES):
        in_maps.append(
            {
                "xT": np.ascontiguousarray(x[c].T.astype(NPBF)),
                "wq": Wq_b,
                "wk": Wk_b,
                "wv": Wv_b,
                "bq": bq_c,
                "bk": bk_c,
                "bvr": bv_r,
            }
        )
    return in_maps


def run(x, Wq, bq, Wk, bk, Wv, bv, **run_kwargs):
    """Run on the 8 NeuronCores; returns (output, BassKernelResults)."""
    from concourse.bass_utils import run_bass_kernel_spmd

    nc = _get_nc()
    in_maps = _make_in_maps(x, Wq, bq, Wk, bk, Wv, bv)
    res = run_bass_kernel_spmd(
        nc, in_maps, core_ids=list(range(N_CORES)), **run_kwargs
    )
    out = np.stack(
        [np.asarray(r["out"]).astype(np.float32) for r in res.results],
        axis=0,
    )
    return out, res


def kernel(x, Wq, bq, Wk, bk, Wv, bv):
    out, _ = run(x, Wq, bq, Wk, bk, Wv, bv)
    return out


# revision 32
# speedup vs baseline: 1.0042x; 1.0031x over previous
"""Single-head attention (B=8, S=2048, D=1024, d_k=512), data-parallel over
batch across 8 NeuronCores, all-bf16 matmul pipeline.

Per-core dataflow (batch element b on core b), everything derived from x^T so
no on-chip transposes are needed anywhere:

  host:  xT = bf16(x[b].T)                            [1024, 2048]
  Q^T = Wq^T x + bq   -> [dk, S]  (k on partitions)   via matmul(lhsT=Wq, rhs=xT)
  K^T = Wk^T x + bk   -> [dk, S]
  V'  = x^T Wv + bv   -> [S, dk]  (s on partitions)   bv added on the DVE
                                                      eviction via a rank-1
                                                      ones (x) bv broadcast
  S^T[s,q] = (K^T)^T-free slices: matmul(lhsT=K^T tile, rhs=Q^T chunk)
  E^T = exp(S^T / sqrt(dk))       (no max subtraction; |scores| < ~4)
  Z[q] = ones^T @ (DVE running sum of E^T tiles)      [1, q]
  out[q,k] = sum_st matmul(lhsT=E^T tile, rhs=V' tile)  (q on partitions)
  out = bf16(pso * (1/Z))         one ACT op per q-block: per-partition
                                  scale AP; 1/Z via 4 PE transposes + DVE
                                  reciprocal on [128,4]. Since sum_s E/Z = 1
                                  the V'-folded bv comes out as out + bv.

Scheduling notes (measured on HW):
- each dma_start costs ~600ns serialized on its issuing sequencer, but one
  DMA is split across all 16 SDMA engines -> few, large DMAs on multiple
  queues (sync + scalar HWDGE rings, gpsimd SWDGE for the tiny biases).
- the first projection chunk runs at N=256 so the first matmul group only
  needs half of xt chunk 0 plus Wq m-blocks 0-1 (one DMA each).
- ~40 tiny warmup matmuls (gated only on a DVE memset) run during the
  initial DMA wait so the HAM clock gate is at 8/8 when the stream starts.
- bf16 streams at the same 1 column/cycle as fp32r but halves DMA bytes and
  LDWEIGHTS time (FWL works for 16-bit, not fp32).

Verified numerics vs fp32 reference: rel_err ~3.9e-3 (budget 2e-2).
"""

import numpy as np
import ml_dtypes

import concourse.bass as bass
import concourse.mybir as mybir
import concourse.tile as tile

B, S, D, DK = 8, 2048, 1024, 512
N_CORES = 8
P = 128
DT = D // P      # 8 d-tiles (contraction tiles for projections)
MT = DK // P     # 4 k-tiles
ST = S // P      # 16 s-tiles
NCH = S // 512   # 4 free-dim chunks of 512
SCALE = float(1.0 / np.sqrt(np.float32(DK)))
N_WARM = 112      # tiny PE warmup matmuls issued under the first DMA wait

F32 = mybir.dt.float32
F32R = mybir.dt.float32r
BF16 = mybir.dt.bfloat16
NPBF = ml_dtypes.bfloat16


def _round_f32r(a):
    """Round fp32 array to fp32r precision (rne at 11 mantissa bits), so the
    device can load it with a plain HWDGE DMA and bitcast to f32r."""
    u = np.ascontiguousarray(a, dtype=np.float32).view(np.uint32).astype(np.uint64)
    sh = np.uint64(12)
    half = np.uint64(1 << 11)
    lsb = (u >> sh) & np.uint64(1)
    r = ((u + half - np.uint64(1) + lsb) >> sh) << sh
    return r.astype(np.uint32).view(np.float32).reshape(a.shape)


def _split_excess_waits(nc, max_waits=1):
    """This walrus build accepts very few sync waits per instruction (and adds
    its own implicit queue waits to Drain). Move excess BIR waits onto
    dedicated NoOps inserted just before the over-subscribed instruction."""
    count = 0
    for f in nc.m.functions:
        for b in f.blocks:
            insts = list(b.instructions)
            out = []
            for ins in insts:
                si = getattr(ins, "sync_info", None)
                waits = list(si.on_wait) if si is not None else []
                cap = 0 if isinstance(ins, mybir.InstDrain) else max_waits
                if len(waits) > cap:
                    keep = waits[len(waits) - cap:] if cap else []
                    excess = waits[: len(waits) - cap]
                    for i in range(0, len(excess), max_waits):
                        chunk = excess[i : i + max_waits]
                        count += 1
                        nop = mybir.InstNoOp(
                            name=f"Wsplit-{count}", engine=ins.engine
                        )
                        nop.sync_info = mybir.SyncInfo(
                            on_wait=chunk, on_update=[]
                        )
                        out.append(nop)
                    ins.sync_info = mybir.SyncInfo(
                        on_wait=keep, on_update=list(si.on_update)
                    )
                out.append(ins)
            live = b.instructions
            live.clear()
            live.extend(out)
    return count


def build_nc(split_waits=True):
    nc = bass.Bass()
    xT = nc.dram_tensor("xT", [D, S], BF16, kind="ExternalInput")
    wq = nc.dram_tensor("wq", [D, DK], BF16, kind="ExternalInput")
    wk = nc.dram_tensor("wk", [D, DK], BF16, kind="ExternalInput")
    wv = nc.dram_tensor("wv", [D, DK], BF16, kind="ExternalInput")
    bq = nc.dram_tensor("bq", [P, MT], F32, kind="ExternalInput")
    bk = nc.dram_tensor("bk", [P, MT], F32, kind="ExternalInput")
    bvr = nc.dram_tensor("bvr", [1, DK], F32, kind="ExternalInput")
    out_d = nc.dram_tensor("out", [S, DK], BF16, kind="ExternalOutput")

    xT_r = xT.rearrange("(dt p) s -> p dt s", p=P)
    wq_r = wq.rearrange("(dt p) k -> p dt k", p=P)
    wk_r = wk.rearrange("(dt p) k -> p dt k", p=P)
    wv_r = wv.rearrange("(dt p) k -> p dt k", p=P)

    with tile.TileContext(nc) as tc:
        with tc.tile_pool(name="persist", bufs=1) as persist:
            qT = persist.tile([P, MT, S], BF16, tag="qT")
            kT = persist.tile([P, MT, S], BF16, tag="kT")
            v_sb = persist.tile([P, ST, DK], BF16, tag="v")
            bq_sb = persist.tile([P, MT], F32, tag="bq")
            bk_sb = persist.tile([P, MT], F32, tag="bk")
            bv_row = persist.tile([1, DK], F32R, tag="bv_row")
            bv_bc = persist.tile([P, DK], F32, tag="bv_bc")
            ones_col = persist.tile([P, 1], F32, tag="ones_col")
            ones_row = persist.tile([1, P], F32, tag="ones_row")
            ones_w = persist.tile([P, 64], F32, tag="ones_w")
            ones_id = persist.tile([1, 1], F32, tag="ones_id")

            nc.gpsimd.dma_start(out=bq_sb, in_=bq[:, :])
            nc.gpsimd.dma_start(out=bk_sb, in_=bk[:, :])
            nc.gpsimd.dma_start(out=bv_row, in_=bvr[:, :].bitcast(F32R))
            # ones via DVE memset only (no ACT dependency: ACT is busy with
            # its table load during the preamble); bitcast to f32r at use
            nc.vector.memset(ones_col, 1.0)
            nc.vector.memset(ones_row, 1.0)
            nc.vector.memset(ones_w, 1.0)
            nc.vector.memset(ones_id, 1.0)

            # ---------- Phase B: projections ----------
            # psC allocated first so it owns banks untouched by the phase-B
            # pools: phase C's first matmul then has no WAR dependency on
            # the last projection eviction
            psC_cm = tc.tile_pool(name="psC", bufs=3, space="PSUM")
            psC = psC_cm.__enter__()
            with tc.tile_pool(name="wpool", bufs=1) as wpool, \
                 tc.tile_pool(name="xpool", bufs=2) as xpool, \
                 tc.tile_pool(name="psW", bufs=1, space="PSUM") as psW:
                # PE warmup during the initial DMA wait: HAM reaches 8/8
                # before the real stream starts
                psw = psW.tile([1, 64], F32, tag="psw")
                for _ in range(N_WARM):
                    nc.tensor.matmul(
                        psw,
                        lhsT=ones_col[:, 0:1].bitcast(F32R),
                        rhs=ones_w.bitcast(F32R),
                        start=True,
                        stop=True,
                    )

                wq_sb = wpool.tile([P, DT, DK], BF16, tag="wq")
                wk_sb = wpool.tile([P, DT, DK], BF16, tag="wk")
                wv_sb = wpool.tile([P, DT, DK], BF16, tag="wv")
                xt0 = xpool.tile([P, DT, 512], BF16, tag="xt")
                # Large DMAs (1KB descriptors), arrival-ordered: scalar
                # ring carries wq, sync ring xt0 then wk/wv, so during the
                # head only (xt0, wq) compete for SDMA engines.
                nc.sync.dma_start(out=xt0[:, 0:4, :], in_=xT_r[:, 0:4, 0:512])
                nc.sync.dma_start(out=xt0[:, 4:8, :], in_=xT_r[:, 4:8, 0:512])
                nc.scalar.dma_start(out=wq_sb[:, 0:4, :], in_=wq_r[:, 0:4, :])
                nc.scalar.dma_start(out=wq_sb[:, 4:8, :], in_=wq_r[:, 4:8, :])
                for w_sb_, w_r_ in ((wk_sb, wk_r), (wv_sb, wv_r)):
                    nc.sync.dma_start(
                        out=w_sb_[:, 0:4, :], in_=w_r_[:, 0:4, :]
                    )
                    nc.sync.dma_start(
                        out=w_sb_[:, 4:8, :], in_=w_r_[:, 4:8, :]
                    )

                psB_cm = tc.tile_pool(name="psB", bufs=4, space="PSUM")
                psB = psB_cm.__enter__()

                def proj_group(wacc, m, xt, cols, out_sb, bias_sb, sc,
                               first=False):
                    ps = psB.tile([P, 512], F32, tag="psb")
                    ps_sl = ps[:, 0 : cols.stop - cols.start]
                    for d in range(DT):
                        if first and d == 4:
                            # second warmup burst inside the first group:
                            # covers the DMA wait for the second halves of
                            # xt0/wq so the HAM clock gate never sees idle
                            for _ in range(97):
                                nc.tensor.matmul(
                                    psw,
                                    lhsT=ones_col[:, 0:1].bitcast(F32R),
                                    rhs=ones_w.bitcast(F32R),
                                    start=True,
                                    stop=True,
                                )
                        nc.tensor.matmul(
                            ps_sl,
                            lhsT=wacc(m, d),
                            rhs=xt[:, d, cols],
                            start=(d == 0),
                            stop=(d == DT - 1),
                        )
                    nc.scalar.activation(
                        out=out_sb[
                            :, m, sc * 512 + cols.start : sc * 512 + cols.stop
                        ],
                        in_=ps_sl,
                        func=mybir.ActivationFunctionType.Identity,
                        bias=bias_sb[:, m : m + 1],
                    )

                def v_group(xt, i, sc):
                    st = sc * 4 + i
                    psv = psB.tile([P, 512], F32, tag="psb")
                    for d in range(DT):
                        nc.tensor.matmul(
                            psv,
                            lhsT=xt[:, d, i * P : (i + 1) * P],
                            rhs=wv_sb[:, d, :],
                            start=(d == 0),
                            stop=(d == DT - 1),
                        )
                    nc.vector.tensor_add(v_sb[:, st, :], psv, bv_bc)

                wq_a = lambda m, d: wq_sb[:, d, m * P : (m + 1) * P]
                wk_a = lambda m, d: wk_sb[:, d, m * P : (m + 1) * P]
                full = slice(0, 512)
                for sc in range(NCH):
                    if sc == 0:
                        xt = xt0
                    else:
                        xt = xpool.tile([P, DT, 512], BF16, tag="xt")
                        nc.sync.dma_start(
                            out=xt, in_=xT_r[:, :, sc * 512 : (sc + 1) * 512]
                        )
                    for m in range(MT):
                        proj_group(wq_a, m, xt, full, qT, bq_sb, sc,
                                   first=(sc == 0 and m == 0))
                        if sc == 0 and m == 0:
                            # bv broadcast [P, DK] via rank-1 ones (x)
                            # bv_row; placed after the first group so the
                            # PE never waits on the slow SWDGE bias DMA
                            psbv = psB.tile([P, 512], F32, tag="psb")
                            nc.tensor.matmul(
                                psbv,
                                lhsT=ones_row[0:1, :].bitcast(F32R),
                                rhs=bv_row[0:1, :],
                                start=True,
                                stop=True,
                            )
                            nc.scalar.copy(bv_bc, psbv)
                    for m in range(MT):
                        proj_group(wk_a, m, xt, full, kT, bk_sb, sc)
                    for i in range(4):
                        v_group(xt, i, sc)
                psB_cm.__exit__(None, None, None)

            # ---------- Phase C: attention ----------
            with tc.tile_pool(name="epool", bufs=3) as epool, \
                 tc.tile_pool(name="spool", bufs=2) as spool, \
                 tc.tile_pool(name="psO", bufs=2, space="PSUM") as psO, \
                 tc.tile_pool(name="psZ", bufs=1, space="PSUM") as psZ:
                for qc in range(NCH):
                    eT = epool.tile([P, ST, 512], BF16, tag="eT")
                    acc_z = spool.tile([P, 512], F32R, tag="acc_z")
                    # S^T tiles: [s-part, 512 q], exp on eviction; running
                    # f32 sum of E^T tiles on DVE for the Z row
                    for st in range(ST):
                        pss = psC.tile([P, 512], F32, tag="pss")
                        for kt in range(MT):
                            nc.tensor.matmul(
                                pss,
                                lhsT=kT[:, kt, st * P : (st + 1) * P],
                                rhs=qT[:, kt, qc * 512 : (qc + 1) * 512],
                                start=(kt == 0),
                                stop=(kt == MT - 1),
                            )
                        nc.scalar.activation(
                            out=eT[:, st, :],
                            in_=pss,
                            func=mybir.ActivationFunctionType.Exp,
                            scale=SCALE,
                        )
                        if st == 0:
                            nc.vector.tensor_copy(acc_z, eT[:, 0, :])
                        else:
                            nc.vector.tensor_add(acc_z, acc_z, eT[:, st, :])
                    # Z chain on the side: reduce acc_z to [1,512], PE-
                    # transpose to q-partitions in 128-col strips, then one
                    # cheap full-width DVE reciprocal on [128,4]. The PE
                    # pieces are interleaved behind qsub 0's PV stream.
                    psz = psZ.tile([1, 512], F32, tag="psz")
                    zrow = spool.tile([1, 512], F32, tag="zrow")
                    zcol_ps = psZ.tile([P, MT], F32, tag="zcol")
                    zrc = spool.tile([P, MT], F32, tag="zrc")

                    last = qc == NCH - 1
                    for j in range(MT):
                        nhalf = 2 if (last and j == MT - 1) else 1
                        for h in range(nhalf):
                            if nhalf == 1:
                                cols = slice(0, DK)
                            else:
                                cols = slice(0, 384) if h == 0 else slice(384, DK)
                            pso_full = psO.tile([P, 512], F32, tag="pso")
                            pso = pso_full[:, 0 : cols.stop - cols.start]
                            for st in range(ST):
                                nc.tensor.matmul(
                                    pso,
                                    lhsT=eT[:, st, j * P : (j + 1) * P],
                                    rhs=v_sb[:, st, cols],
                                    start=(st == 0),
                                    stop=(st == ST - 1),
                                )
                            if j == 0 and h == 0:
                                # Z reduce + transposes, scheduled behind
                                # qsub 0's PV group
                                nc.tensor.matmul(
                                    psz,
                                    lhsT=ones_col[:, 0:1].bitcast(F32R),
                                    rhs=acc_z,
                                    start=True,
                                    stop=True,
                                )
                                nc.scalar.copy(zrow, psz[0:1, :])
                                for t in range(MT):
                                    nc.tensor.transpose(
                                        zcol_ps[:, t : t + 1],
                                        zrow[0:1, t * P : (t + 1) * P],
                                        ones_id[0:1, 0:1],
                                    )
                                nc.vector.reciprocal(zrc, zcol_ps)
                            ow = cols.stop - cols.start
                            oT = spool.tile([P, ow], BF16, tag=f"oT{ow}")
                            nc.scalar.mul(oT, pso, zrc[:, j : j + 1])
                            row0 = qc * 512 + j * P
                            if nhalf == 1:
                                nc.sync.dma_start(
                                    out=out_d[row0 : row0 + P, :], in_=oT
                                )
                            else:
                                # final store split across both HWDGE rings
                                # so only a short chain trails the last MM
                                nc.sync.dma_start(
                                    out=out_d[row0 : row0 + 64, cols],
                                    in_=oT[0:64, :],
                                )
                                nc.scalar.dma_start(
                                    out=out_d[row0 + 64 : row0 + P, cols],
                                    in_=oT[64:P, :],
                                )

            psC_cm.__exit__(None, None, None)

    if split_waits:
        _split_excess_waits(nc)
    return nc


_NC_CACHE = None


def _get_nc():
    global _NC_CACHE
    if _NC_CACHE is None:
        _NC_CACHE = build_nc()
    return _NC_CACHE


def _make_in_maps(x, Wq, bq, Wk, bk, Wv, bv):
    x = np.asarray(x, dtype=np.float32)
    Wq_b = np.ascontiguousarray(np.asarray(Wq, np.float32).astype(NPBF))
    Wk_b = np.ascontiguousarray(np.asarray(Wk, np.float32).astype(NPBF))
    Wv_b = np.ascontiguousarray(np.asarray(Wv, np.float32).astype(NPBF))
    bq_c = np.ascontiguousarray(np.asarray(bq, np.float32).reshape(MT, P).T)
    bk_c = np.ascontiguousarray(np.asarray(bk, np.float32).reshape(MT, P).T)
    bv_r = np.ascontiguousarray(
        _round_f32r(np.asarray(bv, np.float32)).reshape(1, DK)
    )
    in_maps = []
    for c in range(N_CORES):
        in_maps.append(
            {
                "xT": np.ascontiguousarray(x[c].T.astype(NPBF)),
                "wq": Wq_b,
                "wk": Wk_b,
                "wv": Wv_b,
                "bq": bq_c,
                "bk": bk_c,
                "bvr": bv_r,
            }
        )
    return in_maps


def run(x, Wq, bq, Wk, bk, Wv, bv, **run_kwargs):
    """Run on the 8 NeuronCores; returns (output, BassKernelResults)."""
    from concourse.bass_utils import run_bass_kernel_spmd

    nc = _get_nc()
    in_maps = _make_in_maps(x, Wq, bq, Wk, bk, Wv, bv)
    res = run_bass_kernel_spmd(
        nc, in_maps, core_ids=list(range(N_CORES)), **run_kwargs
    )
    out = np.stack(
        [np.asarray(r["out"]).astype(np.float32) for r in res.results],
        axis=0,
    )
    return out, res


def kernel(x, Wq, bq, Wk, bk, Wv, bv):
    out, _ = run(x, Wq, bq, Wk, bk, Wv, bv)
    return out
